# revision 6
# baseline (speedup 1.0000x reference)
"""AdaptiveGridKANLayer on 8 TRN2 NeuronCores.

out[b,o] = sum_i sum_g exp(-((x[b,i]-c_g)/w)^2) * coeffs[o,i,g]
         + sum_i silu(x[b,i]) * base_w[o,i]

B=65536, in=out=128, G=8, centers = linspace(-1,1,8), w = 2/7.

Strategy (data-parallel over batch, weights replicated):
- Host: transpose x to feature-major [128, B] and shard columns 8 ways;
  fold the Gaussian factorization constants into the coeffs.
- Device, with u = (x+1)/w:  basis_g = exp(-(u-g)^2) = p * s^g * e^(g^2-7g)
  where p = exp(-u^2) (ScalarE: Square then Exp) and s = exp(2x/w) = exp(7x)
  (ScalarE: one Exp). The e^(g^2-7g) factor is folded into coeffs on host.
  VectorE builds t_g = t_{g-1} * s (7 bf16 multiplies); TensorE contracts the
  9 K-tiles (8 Gaussian + 1 silu) as [128,128]x[128,512] bf16 matmuls with
  f32 PSUM accumulation; silu(x) comes from one ScalarE pass (separate act
  table, so it runs as a leading phase gated off from the Exp phase).
- Output produced transposed [128, B_shard] and untransposed on host.
"""

import numpy as np

BATCH = 65536
IN_F = 128
OUT_F = 128
GRID = 8
NCORES = 8
BLOC = BATCH // NCORES  # 8192 batch columns per core
FD = 2048  # elementwise chunk (free dim)
NCH = BLOC // FD
MMF = 512  # matmul free dim (one PSUM bank)
NSUB = FD // MMF
W = 2.0 / (GRID - 1)

_NC = None


def _build():
    import concourse.mybir as mybir
    from concourse import bacc
    from concourse.tile import TileContext

    AF = mybir.ActivationFunctionType
    bf16 = mybir.dt.bfloat16
    f32 = mybir.dt.float32

    nc = bacc.Bacc("TRN2", num_devices=NCORES)
    cst = nc.alloc_sbuf_tensor("const-float32-bias-c", [128, 1], f32)
    nc.gpsimd.memset(cst.ap(), 1.0 / W)
    nc.const_aps.aps[(f32, 1.0 / W)] = cst.ap()
    nc.all_engine_barrier()
    xt = nc.dram_tensor("xt", [128, BLOC], f32, kind="ExternalInput").ap()
    wt = nc.dram_tensor("wt", [128, 9 * 128], bf16, kind="ExternalInput").ap()
    out = nc.dram_tensor("out", [128, BLOC], f32, kind="ExternalOutput").ap()

    with TileContext(nc) as tc:
        with (
            tc.tile_pool(name="const", bufs=1) as cpool,
            tc.tile_pool(name="work", bufs=2) as wpool,
            tc.tile_pool(name="psum", bufs=2, space="PSUM") as ppool,
        ):
            w_sb = cpool.tile([128, 9, 128], bf16)
            nc.sync.dma_start(w_sb[:], wt.rearrange("p (g o) -> p g o", g=9))

            x_all = cpool.tile([128, BLOC], f32)
            silu_all = cpool.tile([128, BLOC], bf16)

            # Phase 0: stream x in; silu on ScalarE (silu_and_others table).
            for c in range(NCH):
                cs = slice(c * FD, (c + 1) * FD)
                nc.sync.dma_start(x_all[:, cs], xt[:, cs])
                nc.scalar.activation(silu_all[:, cs], x_all[:, cs], AF.Silu)

            # Bias tiles double as the phase gate: every Exp/Square below
            # reads one of them, and they are written after the last Silu,
            # so the scheduler cannot interleave the two act-table phases.
            bias0 = cpool.tile([128, 1], f32)
            bias_c = cpool.tile([128, 1], f32)
            nc.scalar.activation(
                bias0[:], silu_all[:, BLOC - 1 : BLOC], AF.Identity, scale=0.0
            )
            nc.scalar.activation(
                bias_c[:], bias0[:], AF.Identity, bias=1.0 / W, scale=1.0
            )

            # Phase 1 (exp_and_others table): per chunk build s, p, t-chain,
            # then 9 accumulating matmuls per 512-column subtile.
            for c in range(NCH):
                cs = slice(c * FD, (c + 1) * FD)
                xc = x_all[:, cs]
                s = wpool.tile([128, FD], bf16, tag="s")
                nc.scalar.activation(s[:], xc, AF.Exp, bias=bias0[:], scale=2.0 / W)
                q = wpool.tile([128, FD], f32, tag="q")
                nc.scalar.activation(q[:], xc, AF.Square, bias=bias_c[:], scale=1.0 / W)
                t0 = wpool.tile([128, FD], bf16, tag="t0")
                nc.scalar.activation(t0[:], q[:], AF.Exp, bias=bias0[:], scale=-1.0)
                tg = [t0]
                for g in range(1, GRID):
                    t = wpool.tile([128, FD], bf16, tag=f"t{g}")
                    nc.vector.tensor_mul(t[:], tg[-1][:], s[:])
                    tg.append(t)

                psum = ppool.tile([128, FD], f32)
                for g in range(GRID):
                    for n in range(NSUB):
                        ns = slice(n * MMF, (n + 1) * MMF)
                        nc.tensor.matmul(
                            psum[:, ns], w_sb[:, g, :], tg[g][:, ns],
                            start=(g == 0), stop=False,
                        )
                for n in range(NSUB):
                    ns = slice(n * MMF, (n + 1) * MMF)
                    nc.tensor.matmul(
                        psum[:, ns], w_sb[:, 8, :],
                        silu_all[:, c * FD + n * MMF : c * FD + (n + 1) * MMF],
                        start=False, stop=True,
                    )

                ob = wpool.tile([128, FD], f32, tag="ob")
                nc.vector.tensor_copy(ob[:], psum[:])
                nc.sync.dma_start(out[:, cs], ob[:])

    nc.compile()
    return nc


def _prep_weights(coeffs, base_w):
    import ml_dtypes

    g = np.arange(GRID, dtype=np.float64)
    K = np.exp(7.0 * g - g * g)  # t_g = basis_g * e^(g^2-7g) -> fold inverse
    blocks = [
        (coeffs[:, :, gi].astype(np.float64) * K[gi]).T for gi in range(GRID)
    ]  # [in, out] each
    blocks.append(base_w.astype(np.float64).T)
    wt = np.concatenate(blocks, axis=1)  # [128, 9*128]
    return np.ascontiguousarray(wt.astype(ml_dtypes.bfloat16))


def kernel(x, coeffs, base_w, centers):
    from concourse.bass_utils import run_bass_kernel_spmd

    global _NC
    if _NC is None:
        _NC = _build()

    wt = _prep_weights(coeffs, base_w)
    xT = np.ascontiguousarray(np.asarray(x, dtype=np.float32).T)  # [128, B]
    in_maps = [
        {
            "xt": np.ascontiguousarray(xT[:, c * BLOC : (c + 1) * BLOC]),
            "wt": wt,
        }
        for c in range(NCORES)
    ]
    res = run_bass_kernel_spmd(_NC, in_maps, list(range(NCORES)))
    outT = np.concatenate([res.results[c]["out"] for c in range(NCORES)], axis=1)
    return np.ascontiguousarray(outT.T)
